# revision 24
# baseline (speedup 1.0000x reference)
"""GAT (2-layer + linear head) Bass kernel for Trainium2, 8 NeuronCores.

Strategy (graph/data parallel, per sharding hint):
  - Nodes are sharded by dst range across 8 cores (12544/core after padding
    N=100000 -> 100352).  Every core runs the SAME program; per-core behavior
    comes only from per-core input data.  Node order is ROTATED per core so
    "my shard" is always local tiles 0..97.
  - Phase A (replicated): [h1|asrc1|adst1] = x @ [W1|Asrc|Adst] for all
    nodes -> DRAM gather table (288B rows) + shard-local adst1 table.
  - L1 edge pass (dst-sharded): edges sorted by dst, grouped into 128-node
    dst blocks, padded to a uniform T tiles of 128 edges.  Per block one
    indirect DMA gathers [h1|asrc1] rows by src and one gathers adst1 by
    dst; one-hot masks (is_equal vs iota) turn the segment softmax+sum into
    PSUM-accumulated matmuls.  Pad edges are killed by the mask (their dst
    slot is 999 which matches no iota column).  Self-loops are handled
    separately from contiguous rows (no gather, no mask).
  - Between layers: one AllGather of the fused 17-f32/node layer-2 table
    g2 = [elu(out1+b1) @ (W2@Wh) | .. @ (W2@a_src2') | .. @ (W2@a_dst2')].
  - L2 edge pass mirrors L1 on 68B rows; per-core [12544,16] outputs are
    concatenated on host.

Host does integer index prep (sort/shard/pad/rotate) and exact linear
weight fusion only; all floating-point graph compute runs on device.
"""

import contextlib
import numpy as np

N = 100000
E = 1600000
D = 64
H = 8
C = 8
OUT = 16
NEG_SLOPE = 0.2
NCORES = 8
PB = 128                      # nodes per dst block
PAD_DLOC = 999.0              # pad-edge dst slot: matches no iota column

_cache = {}


def make_cfg(ncores=NCORES, nblk=98, T=18, chunk=1024):
    return dict(
        ncores=ncores,
        nblk=nblk,
        nblk_total=nblk * ncores,
        npad=nblk * ncores * PB,
        shard=nblk * PB,
        T=T,
        chunk=chunk,
    )


# ===================================================================== host
def host_prep(edge_index, cfg):
    """Sort/shard/pad edges; build per-core index arrays (int work only)."""
    npad, shard, nblk, T = cfg["npad"], cfg["shard"], cfg["nblk"], cfg["T"]
    ncores = cfg["ncores"]
    src = np.asarray(edge_index[0], np.int64)
    dst = np.asarray(edge_index[1], np.int64)
    # note: accidental (i,i) edges in the input stay in the edge list; the
    # self path below models only the loop the reference ADDS per node.

    order = np.argsort(dst, kind="stable")
    src, dst = src[order], dst[order]

    nblk_total = cfg["nblk_total"]
    blk = dst // PB
    counts = np.bincount(blk, minlength=nblk_total)
    assert counts.max() <= T * PB, (counts.max(), T)

    starts = np.zeros(nblk_total + 1, np.int64)
    np.cumsum(counts, out=starts[1:])

    src_g = np.full((nblk_total, T * PB), -1, np.int64)
    dloc = np.full((nblk_total, T * PB), -1, np.int64)
    within = np.arange(len(dst)) - starts[blk]
    src_g[blk, within] = src
    dloc[blk, within] = dst % PB
    # slot j -> (tau=j//128, p=j%128)
    src_g = src_g.reshape(nblk_total, T, PB).transpose(0, 2, 1)  # [B,128,T]
    dloc = dloc.reshape(nblk_total, T, PB).transpose(0, 2, 1)

    per_core = []
    for c in range(ncores):
        lo = c * nblk
        sg = src_g[lo:lo + nblk]
        dl = dloc[lo:lo + nblk].astype(np.float32)
        pad = sg < 0
        s1 = (sg - c * shard) % npad       # rotated coords for L1 table
        s1[pad] = 0
        s2 = sg.copy()                     # global coords for L2 table
        s2[pad] = 0
        dg = np.arange(nblk)[:, None, None] * PB + dloc[lo:lo + nblk]
        dg[pad] = 0
        dl[pad] = PAD_DLOC
        per_core.append(dict(
            src1=np.ascontiguousarray(s1.astype(np.int32)),
            src2=np.ascontiguousarray(s2.astype(np.int32)),
            dstg=np.ascontiguousarray(dg.astype(np.int32)),
            dloc=np.ascontiguousarray(dl),
        ))
    return per_core


def fuse_weights(W1, a_src1, a_dst1, b1, W2, a_src2, a_dst2, b2, Wh, bh):
    """Exact linear weight fusion (host)."""
    HC = H * C
    Asrc = np.zeros((HC, H), np.float32)
    Adst = np.zeros((HC, H), np.float32)
    for h in range(H):
        Asrc[h * C:(h + 1) * C, h] = a_src1[h]
        Adst[h * C:(h + 1) * C, h] = a_dst1[h]
    Wcat1 = np.concatenate([W1, W1 @ Asrc, W1 @ Adst], axis=1).astype(np.float32)
    Wg = W2 @ Wh                                   # [64,16]
    Ws = W2 @ a_src2.reshape(C, 1)                 # [64,1]
    Wd = W2 @ a_dst2.reshape(C, 1)                 # [64,1]
    Wcomb2 = np.concatenate([Wg, Ws, Wd], axis=1).astype(np.float32)
    # elu(x) = max(x,0) + exp(min(x,0)) - 1; the "-1 @ Wcomb2" is folded:
    Wcorr2 = (-Wcomb2.sum(axis=0)).astype(np.float32)
    bhh = (b2 @ Wh + bh).astype(np.float32)
    return Wcat1, Wcomb2, Wcorr2, bhh


def build_consts(b1, Wcorr2, bhh):
    consts = np.zeros((128, 354), np.float32)
    consts[:, 0:128] = np.arange(128, dtype=np.float32)[None, :]
    consts[:, 128:256] = np.eye(128, dtype=np.float32)
    consts[:, 256:320] = np.asarray(b1, np.float32)[None, :]
    consts[:, 320:338] = Wcorr2[None, :]
    consts[:, 338:354] = bhh[None, :]
    return consts


def _split_pe_waits(nc, sem):
    """PE is hardware-decoded: a Matmult can encode at most one sync wait.
    Move every matmul's waits onto standalone PE no-ops in front of it.
    Each no-op gets a benign update on a dedicated sem (sim invariant)."""
    import bass_rust
    fn = nc.m.functions[0]
    k = 0
    moved = 0
    for blk in fn.blocks:
        il = blk.instructions
        new = []
        for inst in il:
            si = inst.sync_info
            nw = len(si.on_wait) if si is not None else 0
            is_mm = type(inst).__name__ == "InstMatmult"
            if si is not None and (nw >= 2 or (is_mm and nw >= 1)):
                for w in si.on_wait:
                    nop = bass_rust.InstNoOp(
                        name=f"I-pewait-{k}", engine=inst.engine,
                        text_hint="pewait")
                    nop.sync_info = bass_rust.SyncInfo(
                        on_wait=[w],
                        on_update=[bass_rust.SyncUpdate(
                            sync_type="semaphore", id=sem.num,
                            ant_name=sem.name, update_mode="sem-inc",
                            update_value=1)])
                    new.append(nop)
                    k += 1
                inst.sync_info = bass_rust.SyncInfo(
                    on_wait=[], on_update=list(si.on_update))
                moved += 1
            new.append(inst)
        il[:] = new
    return moved


# =================================================================== device
def build_program(cfg, profile_no_cc=False, split=None):
    # split=1: phase A + L1 only (g2loc/adst2t become outputs)
    # split=2: L2 only (g2 table + adst2 come in as inputs)
    import concourse.bass as bass
    import concourse.mybir as mybir
    import concourse.tile as tile

    f32 = mybir.dt.float32
    i32 = mybir.dt.int32
    AF = mybir.ActivationFunctionType
    OP = mybir.AluOpType

    npad, shard, nblk, T = cfg["npad"], cfg["shard"], cfg["nblk"], cfg["T"]
    chunk = cfg["chunk"]
    ncores = cfg["ncores"]
    assert npad % chunk == 0 and chunk % 256 == 0
    half = chunk // 2
    nsub = half // PB
    nchunk = npad // chunk

    nc = bass.Bass()

    xTi = nc.dram_tensor("xTi", [128, npad // 2], f32, kind="ExternalInput")
    Wcat1 = nc.dram_tensor("Wcat1", [128, 80], f32, kind="ExternalInput")
    Wcomb2 = nc.dram_tensor("Wcomb2", [D, 18], f32, kind="ExternalInput")
    consts = nc.dram_tensor("consts", [128, 354], f32, kind="ExternalInput")
    src1_d = nc.dram_tensor("src1", [nblk, PB, T], i32, kind="ExternalInput")
    src2_d = nc.dram_tensor("src2", [nblk, PB, T], i32, kind="ExternalInput")
    dstg_d = nc.dram_tensor("dstg", [nblk, PB, T], i32, kind="ExternalInput")
    dloc_d = nc.dram_tensor("dloc", [nblk, PB, T], f32, kind="ExternalInput")
    out_d = nc.dram_tensor("out", [shard, OUT], f32, kind="ExternalOutput")

    gtab1 = nc.dram_tensor("gtab1", [npad, 72], f32)
    adst1t = nc.dram_tensor("adst1t", [shard, 8], f32)
    if split == 2:
        g2loc = nc.dram_tensor("g2loc", [shard, 17], f32,
                               kind="ExternalInput")
    else:
        okind = dict(kind="ExternalOutput") if split == 1 else {}
        g2loc = nc.dram_tensor("g2loc", [shard, 17], f32, **okind)
    if split == 2:
        adst2t = nc.dram_tensor("adst2t", [shard, 1], f32,
                                kind="ExternalInput")
        g2ag = nc.dram_tensor("g2ag", [npad, 17], f32, kind="ExternalInput")
    else:
        adst2t = nc.dram_tensor("adst2t", [shard, 1], f32, **okind)
        if split is None:
            g2ag = nc.dram_tensor("g2ag", [npad, 17], f32,
                                  addr_space="Shared")

    cc_sem = nc.alloc_semaphore(name="cc_sem")
    pewait_sem = nc.alloc_semaphore(name="pewait_sem")

    if split == 2:
        _build_l2(nc, cfg, consts, src2_d, dstg_d, dloc_d, out_d, g2loc,
                  adst2t, g2ag, f32, i32, AF, OP)
        _split_pe_waits(nc, pewait_sem)
        return nc

    with tile.TileContext(nc) as tc, contextlib.ExitStack() as es:
        cpool = es.enter_context(tc.tile_pool(name="consts", bufs=1))
        iota = cpool.tile([128, 128], f32)
        eye = cpool.tile([128, 128], f32)
        b1b = cpool.tile([128, 64], f32)
        wc2b = cpool.tile([128, 18], f32)
        w1s = cpool.tile([128, 80], f32)   # Wcat1 duplicated in both halves
        w2s = cpool.tile([64, 18], f32)
        adst1_sb = cpool.tile([128, nblk * 8], f32)
        nc.sync.dma_start(out=iota[:], in_=consts[:, 0:128])
        nc.sync.dma_start(out=eye[:], in_=consts[:, 128:256])
        nc.sync.dma_start(out=b1b[:], in_=consts[:, 256:320])
        nc.sync.dma_start(out=wc2b[:], in_=consts[:, 320:338])
        nc.sync.dma_start(out=w1s[:], in_=Wcat1[:])
        nc.sync.dma_start(out=w2s[:], in_=Wcomb2[:])

        # ------------------------------------------------------- phase A
        with tc.tile_pool(name="pha", bufs=3) as apool, \
             tc.tile_pool(name="phaps", bufs=4, space="PSUM") as apsum:
            for ch in range(nchunk):
                xt = apool.tile([128, half], f32, tag="xchunk")
                nc.sync.dma_start(
                    out=xt[:],
                    in_=xTi[:, ch * half:(ch + 1) * half])
                for s in range(2 * nsub):
                    a, ss = divmod(s, nsub)
                    t = ch * (2 * nsub) + a * nsub + ss
                    ps = apsum.tile([128, 80], f32, tag="aps")
                    lhsT = xt[a * 64:(a + 1) * 64, ss * PB:(ss + 1) * PB]
                    nc.tensor.matmul(out=ps[:], lhsT=lhsT,
                                     rhs=w1s[a * 64:(a + 1) * 64, :],
                                     start=True, stop=True)
                    grow = apool.tile([128, 80], f32, tag="arow")
                    nc.vector.tensor_copy(out=grow[:], in_=ps[:])
                    nc.sync.dma_start(out=gtab1[t * PB:(t + 1) * PB, :],
                                      in_=grow[:, 0:72])
                    if t < nblk:
                        nc.sync.dma_start(
                            out=adst1t[t * PB:(t + 1) * PB, :],
                            in_=grow[:, 72:80])
                        nc.vector.tensor_copy(
                            out=adst1_sb[:, t * 8:(t + 1) * 8],
                            in_=ps[:, 72:80])

        # ------------------------------------------------------- L1 edges
        with tc.tile_pool(name="l1", bufs=2) as lp, \
             tc.tile_pool(name="l1ps", bufs=2, space="PSUM") as lps, \
             tc.tile_pool(name="l1ps2", bufs=2, space="PSUM") as lps2:
            for b in range(nblk):
                si = lp.tile([128, T], i32, tag="si")
                di = lp.tile([128, T], i32, tag="di")
                dl = lp.tile([128, T], f32, tag="dl")
                nc.sync.dma_start(out=si[:], in_=src1_d[b])
                nc.sync.dma_start(out=di[:], in_=dstg_d[b])
                nc.sync.dma_start(out=dl[:], in_=dloc_d[b])
                grow = lp.tile([128, T, 72], f32, tag="grow")
                gad = lp.tile([128, T, 8], f32, tag="gad")
                for tau in range(T):
                    nc.gpsimd.indirect_dma_start(
                        out=grow[:, tau, :], out_offset=None, in_=gtab1[:],
                        in_offset=bass.IndirectOffsetOnAxis(
                            ap=si[:, tau:tau + 1], axis=0))
                    nc.gpsimd.indirect_dma_start(
                        out=gad[:, tau, :], out_offset=None, in_=adst1t[:],
                        in_offset=bass.IndirectOffsetOnAxis(
                            ap=di[:, tau:tau + 1], axis=0))
                selfr = lp.tile([128, 72], f32, tag="selfr")
                nc.sync.dma_start(out=selfr[:],
                                  in_=gtab1[b * PB:(b + 1) * PB, :])
                mask = lp.tile([128, T, 128], f32, tag="mask")
                for tau in range(T):
                    nc.vector.tensor_scalar(
                        out=mask[:, tau, :], in0=iota[:],
                        scalar1=dl[:, tau:tau + 1], scalar2=None,
                        op0=OP.is_equal)
                e8 = lp.tile([128, T, 8], f32, tag="e8")
                t8 = lp.tile([128, T, 8], f32, tag="t8")
                nc.vector.tensor_tensor(out=e8[:], in0=grow[:, :, 64:72],
                                        in1=gad[:], op=OP.add)
                nc.vector.tensor_scalar(out=t8[:], in0=e8[:],
                                        scalar1=NEG_SLOPE, scalar2=None,
                                        op0=OP.mult)
                nc.vector.tensor_tensor(out=e8[:], in0=e8[:], in1=t8[:],
                                        op=OP.max)
                nc.scalar.activation(out=grow[:, :, 64:72], in_=e8[:],
                                     func=AF.Exp)
                nc.vector.tensor_tensor(
                    out=grow[:, :, 0:64].rearrange("p t (h c) -> p t h c", c=8),
                    in0=grow[:, :, 0:64].rearrange("p t (h c) -> p t h c", c=8),
                    in1=grow[:, :, 64:72].unsqueeze(3)
                        .to_broadcast([128, T, 8, 8]),
                    op=OP.mult)
                ps = lps.tile([128, 72], f32, tag="psblk")
                for tau in range(T):
                    nc.tensor.matmul(out=ps[:], lhsT=mask[:, tau, :],
                                     rhs=grow[:, tau, :],
                                     start=(tau == 0), stop=(tau == T - 1))
                # self loops
                se = lp.tile([128, 8], f32, tag="se")
                st = lp.tile([128, 8], f32, tag="st")
                nc.vector.tensor_tensor(out=se[:], in0=selfr[:, 64:72],
                                        in1=adst1_sb[:, b * 8:(b + 1) * 8],
                                        op=OP.add)
                nc.vector.tensor_scalar(out=st[:], in0=se[:],
                                        scalar1=NEG_SLOPE, scalar2=None,
                                        op0=OP.mult)
                nc.vector.tensor_tensor(out=se[:], in0=se[:], in1=st[:],
                                        op=OP.max)
                nc.scalar.activation(out=se[:], in_=se[:], func=AF.Exp)
                sw = lp.tile([128, 64], f32, tag="sw")
                nc.vector.tensor_tensor(
                    out=sw[:].rearrange("p (h c) -> p h c", c=8),
                    in0=selfr[:, 0:64].rearrange("p (h c) -> p h c", c=8),
                    in1=se[:].unsqueeze(2).to_broadcast([128, 8, 8]),
                    op=OP.mult)
                nc.vector.tensor_tensor(out=ps[:, 0:64], in0=ps[:, 0:64],
                                        in1=sw[:], op=OP.add)
                nc.vector.tensor_tensor(out=ps[:, 64:72], in0=ps[:, 64:72],
                                        in1=se[:], op=OP.add)
                # normalize + b1 + elu -> h2 ; then g2 row build
                rec = lp.tile([128, 8], f32, tag="rec")
                nc.vector.tensor_scalar(out=rec[:], in0=ps[:, 64:72],
                                        scalar1=1e-16, scalar2=None,
                                        op0=OP.add)
                nc.vector.reciprocal(out=rec[:], in_=rec[:])
                o1 = lp.tile([128, 64], f32, tag="o1")
                nc.vector.tensor_tensor(
                    out=o1[:].rearrange("p (h c) -> p h c", c=8),
                    in0=ps[:, 0:64].rearrange("p (h c) -> p h c", c=8),
                    in1=rec[:].unsqueeze(2).to_broadcast([128, 8, 8]),
                    op=OP.mult)
                nc.vector.tensor_tensor(out=o1[:], in0=o1[:], in1=b1b[:],
                                        op=OP.add)
                mx = lp.tile([128, 64], f32, tag="mx")
                nc.vector.tensor_scalar(out=mx[:], in0=o1[:], scalar1=0.0,
                                        scalar2=None, op0=OP.max)
                nc.vector.tensor_scalar(out=o1[:], in0=o1[:], scalar1=0.0,
                                        scalar2=None, op0=OP.min)
                nc.scalar.activation(out=o1[:], in_=o1[:], func=AF.Exp)
                nc.vector.tensor_tensor(out=mx[:], in0=mx[:], in1=o1[:],
                                        op=OP.add)
                pt = lps2.tile([64, 128], f32, tag="pt")
                nc.tensor.transpose(out=pt[:], in_=mx[:], identity=eye[:])
                h2t = lp.tile([64, 128], f32, tag="h2t")
                nc.vector.tensor_copy(out=h2t[:], in_=pt[:])
                pg = lps2.tile([128, 18], f32, tag="pg")
                nc.tensor.matmul(out=pg[:], lhsT=h2t[:], rhs=w2s[:],
                                 start=True, stop=True)
                g2 = lp.tile([128, 18], f32, tag="g2")
                nc.vector.tensor_tensor(out=g2[:], in0=pg[:], in1=wc2b[:],
                                        op=OP.add)
                nc.sync.dma_start(out=g2loc[b * PB:(b + 1) * PB, :],
                                  in_=g2[:, 0:17])
                nc.sync.dma_start(out=adst2t[b * PB:(b + 1) * PB, :],
                                  in_=g2[:, 17:18])

    if split == 1:
        _split_pe_waits(nc, pewait_sem)
        return nc

    # --------------------------------------------------- collective exchange
    import concourse.mybir as mb

    with nc.Block() as block:
        if profile_no_cc:
            # cost-model profiling build: TimelineSim can't simulate
            # collectives; substitute a local copy of equivalent volume.
            @block.gpsimd
            def _(gp):
                gp.dma_start(out=g2ag[0:shard, :], in_=g2loc[:]).then_inc(
                    cc_sem, 16)
                gp.wait_ge(cc_sem, 16)
        else:
            @block.gpsimd
            def _(gp):
                gp.collective_compute(
                    "AllGather", mb.AluOpType.bypass,
                    replica_groups=[list(range(ncores))],
                    ins=[g2loc[:]],
                    outs=[g2ag[:]],
                ).then_inc(cc_sem)
                gp.wait_ge(cc_sem, 1)
    nc.all_engine_barrier()

    _build_l2(nc, cfg, consts, src2_d, dstg_d, dloc_d, out_d, g2loc,
              adst2t, g2ag, f32, i32, AF, OP)
    _split_pe_waits(nc, pewait_sem)
    return nc


def _build_l2(nc, cfg, consts, src2_d, dstg_d, dloc_d, out_d, g2loc,
              adst2t, g2ag, f32, i32, AF, OP):
    import concourse.bass as bass
    import concourse.tile as tile
    npad, shard, nblk, T = cfg["npad"], cfg["shard"], cfg["nblk"], cfg["T"]

    # --------------------------------------------------------- L2 edge pass
    with tile.TileContext(nc) as tc, contextlib.ExitStack() as es:
        cp2 = es.enter_context(tc.tile_pool(name="c2", bufs=1))
        iota2 = cp2.tile([128, 128], f32)
        bhh2 = cp2.tile([128, 16], f32)
        nc.sync.dma_start(out=iota2[:], in_=consts[:, 0:128])
        nc.sync.dma_start(out=bhh2[:], in_=consts[:, 338:354])

        with tc.tile_pool(name="l2", bufs=2) as lp, \
             tc.tile_pool(name="l2ps", bufs=2, space="PSUM") as lps:
            for b in range(nblk):
                si = lp.tile([128, T], i32, tag="si2")
                di = lp.tile([128, T], i32, tag="di2")
                dl = lp.tile([128, T], f32, tag="dl2")
                nc.sync.dma_start(out=si[:], in_=src2_d[b])
                nc.sync.dma_start(out=di[:], in_=dstg_d[b])
                nc.sync.dma_start(out=dl[:], in_=dloc_d[b])
                g = lp.tile([128, T, 17], f32, tag="g2row")
                gad = lp.tile([128, T, 1], f32, tag="gad2")
                for tau in range(T):
                    nc.gpsimd.indirect_dma_start(
                        out=g[:, tau, :], out_offset=None, in_=g2ag[:],
                        in_offset=bass.IndirectOffsetOnAxis(
                            ap=si[:, tau:tau + 1], axis=0))
                    nc.gpsimd.indirect_dma_start(
                        out=gad[:, tau, :], out_offset=None, in_=adst2t[:],
                        in_offset=bass.IndirectOffsetOnAxis(
                            ap=di[:, tau:tau + 1], axis=0))
                selfr = lp.tile([128, 17], f32, tag="selfr2")
                nc.sync.dma_start(out=selfr[:],
                                  in_=g2loc[b * PB:(b + 1) * PB, :])
                sad = lp.tile([128, 1], f32, tag="sad2")
                nc.sync.dma_start(out=sad[:],
                                  in_=adst2t[b * PB:(b + 1) * PB, :])
                mask = lp.tile([128, T, 128], f32, tag="mask2")
                for tau in range(T):
                    nc.vector.tensor_scalar(
                        out=mask[:, tau, :], in0=iota2[:],
                        scalar1=dl[:, tau:tau + 1], scalar2=None,
                        op0=OP.is_equal)
                e1 = lp.tile([128, T, 1], f32, tag="e1")
                t1 = lp.tile([128, T, 1], f32, tag="t1")
                nc.vector.tensor_tensor(out=e1[:], in0=g[:, :, 16:17],
                                        in1=gad[:], op=OP.add)
                nc.vector.tensor_scalar(out=t1[:], in0=e1[:],
                                        scalar1=NEG_SLOPE, scalar2=None,
                                        op0=OP.mult)
                nc.vector.tensor_tensor(out=e1[:], in0=e1[:], in1=t1[:],
                                        op=OP.max)
                nc.scalar.activation(out=g[:, :, 16:17], in_=e1[:],
                                     func=AF.Exp)
                nc.vector.tensor_tensor(
                    out=g[:, :, 0:16],
                    in0=g[:, :, 0:16],
                    in1=g[:, :, 16:17].to_broadcast([128, T, 16]),
                    op=OP.mult)
                ps = lps.tile([128, 17], f32, tag="psblk2")
                for tau in range(T):
                    nc.tensor.matmul(out=ps[:], lhsT=mask[:, tau, :],
                                     rhs=g[:, tau, :],
                                     start=(tau == 0), stop=(tau == T - 1))
                se = lp.tile([128, 1], f32, tag="se2")
                st = lp.tile([128, 1], f32, tag="st2")
                nc.vector.tensor_tensor(out=se[:], in0=selfr[:, 16:17],
                                        in1=sad[:], op=OP.add)
                nc.vector.tensor_scalar(out=st[:], in0=se[:],
                                        scalar1=NEG_SLOPE, scalar2=None,
                                        op0=OP.mult)
                nc.vector.tensor_tensor(out=se[:], in0=se[:], in1=st[:],
                                        op=OP.max)
                nc.scalar.activation(out=se[:], in_=se[:], func=AF.Exp)
                sw = lp.tile([128, 16], f32, tag="sw2")
                nc.vector.tensor_tensor(out=sw[:], in0=selfr[:, 0:16],
                                        in1=se[:].to_broadcast([128, 16]),
                                        op=OP.mult)
                nc.vector.tensor_tensor(out=ps[:, 0:16], in0=ps[:, 0:16],
                                        in1=sw[:], op=OP.add)
                nc.vector.tensor_tensor(out=ps[:, 16:17], in0=ps[:, 16:17],
                                        in1=se[:], op=OP.add)
                rec = lp.tile([128, 1], f32, tag="rec2")
                nc.vector.tensor_scalar(out=rec[:], in0=ps[:, 16:17],
                                        scalar1=1e-16, scalar2=None,
                                        op0=OP.add)
                nc.vector.reciprocal(out=rec[:], in_=rec[:])
                o = lp.tile([128, 16], f32, tag="o2")
                nc.vector.tensor_tensor(out=o[:], in0=ps[:, 0:16],
                                        in1=rec[:].to_broadcast([128, 16]),
                                        op=OP.mult)
                nc.vector.tensor_tensor(out=o[:], in0=o[:], in1=bhh2[:],
                                        op=OP.add)
                nc.sync.dma_start(out=out_d[b * PB:(b + 1) * PB, :],
                                  in_=o[:])


def build_in_maps(inputs, cfg):
    """Per-core input dict list from full inputs (host prep)."""
    npad, shard = cfg["npad"], cfg["shard"]
    ncores = cfg["ncores"]
    x = np.asarray(inputs["x"], np.float32)
    per_core = host_prep(inputs["edge_index"], cfg)
    Wcat1, Wcomb2, Wcorr2, bhh = fuse_weights(
        np.asarray(inputs["W1"], np.float32),
        np.asarray(inputs["a_src1"], np.float32),
        np.asarray(inputs["a_dst1"], np.float32),
        np.asarray(inputs["b1"], np.float32),
        np.asarray(inputs["W2"], np.float32),
        np.asarray(inputs["a_src2"], np.float32),
        np.asarray(inputs["a_dst2"], np.float32),
        np.asarray(inputs["b2"], np.float32),
        np.asarray(inputs["Wh"], np.float32),
        np.asarray(inputs["bh"], np.float32))
    consts = build_consts(np.asarray(inputs["b1"], np.float32), Wcorr2, bhh)

    n = x.shape[0]
    xpadT = np.zeros((D, npad), np.float32)
    xpadT[:, :n] = x.T
    chunk = cfg["chunk"]
    half = chunk // 2
    nchunk = npad // chunk
    Wcat1d = np.concatenate([Wcat1, Wcat1], axis=0)

    in_maps = []
    for c in range(ncores):
        xTr = np.roll(xpadT, -c * shard, axis=1)
        # interleave: xTi[a*64+f, ch*half+n] = xTr[f, ch*chunk+a*half+n]
        xTi = (xTr.reshape(D, nchunk, 2, half).transpose(2, 0, 1, 3)
               .reshape(128, nchunk * half))
        pc = per_core[c]
        in_maps.append(dict(
            xTi=np.ascontiguousarray(xTi),
            Wcat1=Wcat1d, Wcomb2=Wcomb2, consts=consts,
            src1=pc["src1"], src2=pc["src2"], dstg=pc["dstg"],
            dloc=pc["dloc"],
        ))
    return in_maps


# ==================================================================== entry
def prepare(inputs):
    """Build (nc, in_maps, cfg) for the given full inputs."""
    dst = np.asarray(inputs["edge_index"][1], np.int64)
    n = np.asarray(inputs["x"]).shape[0]
    cnts = np.bincount(dst // PB, minlength=(n + PB - 1) // PB)
    T = max(1, int(-(-cnts.max() // PB)))
    cfg = make_cfg(T=T)

    key = ("prog", T)
    if key not in _cache:
        _cache[key] = build_program(cfg)
    nc = _cache[key]
    in_maps = build_in_maps(inputs, cfg)
    return nc, in_maps, cfg


USE_SPLIT = False  # two launches with host-side AllGather (collective-free)


def run_split(inputs):
    from concourse.bass_utils import run_bass_kernel_spmd
    dst = np.asarray(inputs["edge_index"][1], np.int64)
    n = np.asarray(inputs["x"]).shape[0]
    cnts = np.bincount(dst // PB, minlength=(n + PB - 1) // PB)
    T = max(1, int(-(-cnts.max() // PB)))
    cfg = make_cfg(T=T)
    ncores, shard, npad = cfg["ncores"], cfg["shard"], cfg["npad"]

    k1 = ("prog1", T)
    if k1 not in _cache:
        _cache[k1] = build_program(cfg, split=1)
    k2 = ("prog2", T)
    if k2 not in _cache:
        _cache[k2] = build_program(cfg, split=2)
    nc1, nc2 = _cache[k1], _cache[k2]

    in_maps = build_in_maps(inputs, cfg)
    res1 = run_bass_kernel_spmd(nc1, in_maps, list(range(ncores)))
    g2full = np.concatenate(
        [res1.results[c]["g2loc"] for c in range(ncores)], axis=0)
    in_maps2 = []
    for c in range(ncores):
        m = in_maps[c]
        in_maps2.append(dict(
            consts=m["consts"], src2=m["src2"], dstg=m["dstg"],
            dloc=m["dloc"],
            g2ag=g2full,
            g2loc=np.ascontiguousarray(g2full[c * shard:(c + 1) * shard]),
            adst2t=np.ascontiguousarray(res1.results[c]["adst2t"]),
        ))
    res2 = run_bass_kernel_spmd(nc2, in_maps2, list(range(ncores)))
    out = np.concatenate(
        [res2.results[c]["out"] for c in range(ncores)], axis=0)
    return out[:n]


def kernel(x, edge_index, W1, a_src1, a_dst1, b1, W2, a_src2, a_dst2, b2,
           Wh, bh):
    from concourse.bass_utils import run_bass_kernel_spmd

    inputs = dict(x=x, edge_index=edge_index, W1=W1, a_src1=a_src1,
                  a_dst1=a_dst1, b1=b1, W2=W2, a_src2=a_src2,
                  a_dst2=a_dst2, b2=b2, Wh=Wh, bh=bh)
    if USE_SPLIT:
        out = run_split(inputs)
        return np.ascontiguousarray(
            out[:np.asarray(x).shape[0]].astype(np.float32))
    nc, in_maps, cfg = prepare(inputs)
    res = run_bass_kernel_spmd(nc, in_maps, list(range(cfg["ncores"])))
    out = np.concatenate(
        [res.results[c]["out"] for c in range(cfg["ncores"])], axis=0)
    return np.ascontiguousarray(out[:np.asarray(x).shape[0]].astype(np.float32))
